# revision 1
# baseline (speedup 1.0000x reference)
"""Trainium2 Bass kernel for nn_Attention_65128884077225.

Math: the reference module broadcasts scores [B,H,S,1] along the softmax
axis, so every softmax row is constant -> attention weights are exactly
uniform (1/S). Hence z = mean_s(v) broadcast over s, and the whole module
collapses to, per batch b:

    c[b] = (mean_s x[b,s,:]) @ Wv @ Wout + (bv @ Wout + bout)
    out[b,s,:] = c[b]                      (constant across s)

where Wv = qkv_w[:, 2E:3E], bv = qkv_b[2E:3E].

Sharding (TP-style partial sums, per the hint's tensor-parallel option):
8 cores = 4 batches x 2 sequence-halves. Core c reads rows
[h*1024, (h+1)*1024) of x[b], b=c//2, h=c%2, computes its partial
output row (the mean splits as sum of half-sums / S), and writes the
full-shape partial out[b] in fp16. The host gather sums the two
partials per batch in fp32 (the TP unshard step; rel-err ~1e-3, well
under the 2e-2 gate). The bias enters via core h=0 only.

Device kernel per core (single HWDGE queue so tiles arrive in order):
  - 4 loads of x row-tile pairs (fp32, 4 KiB descriptors) stream
    back-to-back on the sync ring; the fp16 folded weight (and bias,
    when nonzero) follow after the last x tile so they never delay it,
  - a DVE add-chain accumulates tiles t0..t7 while the stream runs;
    4 full-width rank-reduction matmuls on the early tiles ramp the PE
    clock to 2.4 GHz (HAM) for the tail,
  - 4 matmuls vs a 1/S-vector give column part-sums -> xsum^T/S [128,4]
    (1/2048 is a power of two: exact in fp32, and it keeps the
    unscaled fp16 Wc out of subnormal range),
  - DVE casts PSUM->SBUF fp16,
  - fused crow+broadcast(+bias): 4 fp16 matmuls with the xmean chunk
    replicated across 128 lhsT columns (stride-0) accumulate
    xmean @ Wc into every partition of a [128,512] PSUM tile,
  - DVE PSUM->SBUF fp16 cast, then 2 stores with stride-0 source cover
    the [2048, 512] fp16 per-core partial output.

Host only: fold Wc = Wv @ Wout and bc = bv @ Wout + bout (tiny host
GEMM, fp16 cast), shard inputs, sum + fp32-cast per-core partials.
"""

import sys

import numpy as np

if "/opt/trn_rl_repo" not in sys.path and not any(
    p.endswith("trn_rl_repo") for p in sys.path
):
    sys.path.insert(0, "/opt/trn_rl_repo")

import concourse.bacc as bacc
import concourse.mybir as mybir
import concourse.tile as tile
from concourse.bass_utils import run_bass_kernel_spmd

B, S, E = 4, 2048, 512
N_CORES = 8
P = 128
N_T = S // P           # 16 row-tiles of [128, 512] in the full sequence
SH = S // 2            # 1024 input rows per core (half the sequence)
N_HT = SH // P         # 8 row-tiles per core
FP32 = mybir.dt.float32
FP16 = mybir.dt.float16

_CACHE = {}


def build(debug=False, bias=True):
    """Build + compile the per-core Bass program (same for every core)."""
    key = ("dbg" if debug else "nc") + ("" if bias else "_nb")
    if key in _CACHE:
        return _CACHE[key]
    nc = bacc.Bacc(None, target_bir_lowering=False, enable_partition_id=False)
    x_d = nc.dram_tensor("x", [SH, E], FP32, kind="ExternalInput")
    wc_d = nc.dram_tensor("wc", [E, E], FP16, kind="ExternalInput")
    bc_d = nc.dram_tensor("bc", [E], FP16, kind="ExternalInput") if bias else None
    o_d = nc.dram_tensor("o", [S, E], FP16, kind="ExternalOutput")
    if debug:
        dacc_d = nc.dram_tensor("dacc", [P, E], FP32, kind="ExternalOutput")
        dxst_d = nc.dram_tensor("dxst", [P, 4], FP16, kind="ExternalOutput")
        dstk_d = nc.dram_tensor("dstk", [2, E], FP16, kind="ExternalOutput")

    with tile.TileContext(nc) as tc:
        with (
            tc.tile_pool(name="xp", bufs=9) as xp,
            tc.tile_pool(name="wp", bufs=1) as wp,
            tc.tile_pool(name="sp", bufs=1) as sp,
            tc.tile_pool(name="ps", bufs=1, space="PSUM") as ps,
        ):
            # the 1/S mean fold rides the reduction matmul (1/2048 is a
            # power of two, exact in fp32) and keeps the unscaled fp16 Wc
            # out of subnormal range
            ones_col = sp.tile([P, 1], FP32, tag="ones_col")
            nc.vector.memset(ones_col[:], 1.0 / S)
            ones16 = sp.tile([P, 1], FP16, tag="ones16")
            nc.vector.memset(ones16[:], 1.0 / S)
            if bias:
                ones2 = sp.tile([2, P], FP16, tag="ones2")
                nc.vector.memset(ones2[:], 1.0)

            # x arrives as row tiles: partition p holds rows 8p+t (the
            # reduction is permutation-invariant so any row->partition
            # assignment works; pairs give 4 KiB contiguous descriptors).
            # Pair granularity balances chain start (first sem early)
            # against the last DMA's completion straggler; coarser and
            # finer groupings both measured equal or worse.
            # t6/t7 load as singles: their completion sems fire before the
            # (add-bound) chain reaches them, so the last adds never stall
            # on the DMA-completion straggler. Head singles measured worse:
            # the extra issues delay the downstream pairs and re-introduce
            # mid-chain stalls.
            # tiny primer on the idle scalar ring warms the SDMA/HBM path
            # before the real stream (the first DMAs otherwise ramp slowly
            # and their completion sems lag ~0.6us extra); zero displacement
            # of the sync-queue x stream
            primer = sp.tile([4, E], FP16, tag="primer")
            nc.scalar.dma_start(primer[:], wc_d[0:4, :])

            x_pt = x_d.rearrange("(p t) e -> p t e", t=N_HT)
            groups = [(0, 2), (2, 4), (4, 6), (6, 7), (7, 8)]
            tiles = []
            for lo, hi in groups:
                xc = xp.tile([P, hi - lo, E], FP32, tag="xc")
                nc.sync.dma_start(xc[:], x_pt[:, lo:hi, :])
                for i in range(hi - lo):
                    tiles.append(xc[:, i, :])

            # tiny bias then the fp16 folded weight, after the x stream so
            # they never rate-share with (and delay) the last x tiles
            if bias:
                bcr = sp.tile([1, E], FP16, tag="bcr")
                nc.sync.dma_start(bcr[:], bc_d[None, :])
            wcb = wp.tile([P, 4, E], FP16, tag="wcb")
            nc.sync.dma_start(wcb[:], wc_d.rearrange("(k p) e -> p k e", p=P))

            # PE warm-up (HAM): sustained full-width work on the early tiles
            # ramps the clock to 2.4 GHz (small/sparse warms fail to) so the
            # tail matmuls run at full clock.
            p_warm = ps.tile([1, E], FP32, tag="warm")
            for t in (0, 1, 2, 3):
                nc.tensor.matmul(
                    p_warm[:], ones_col[:], tiles[t], start=True, stop=True
                )

            # serial accumulate t0..t7 on DVE, pipelined with the stream
            # (full-width adds: narrow DVE ops pay a large fixed cost, a
            # [128,256] RMW add measures ~830ns vs 690ns for [128,512];
            # folding t7 into the colsum instead is a net loss because
            # fp32 matmuls run as 2 half-speed passes)
            acc = sp.tile([P, E], FP32, tag="acc")
            nc.vector.tensor_add(acc[:], tiles[0], tiles[1])
            for t in range(2, N_HT - 1):
                nc.vector.tensor_add(acc[:], acc[:], tiles[t])
            # the final add casts the finished sum to fp16 (one rounding,
            # ~5e-4 rel): the colsum matmuls then run single-pass with the
            # automatic fast-weight-load path instead of 2x fp32 passes
            acc16 = sp.tile([P, E], FP16, tag="acc16")
            nc.vector.tensor_add(acc16[:], acc[:], tiles[N_HT - 1])

            # column sums -> xsum^T/S [128,4] in PSUM
            # (NB: PSUM start=True resets has_written for the whole bank, so
            # only self-contained or strictly consecutive groups are safe)
            p_red = ps.tile([P, 4], FP32, tag="red")
            for c in range(4):
                nc.tensor.matmul(
                    p_red[:, c : c + 1],
                    acc16[:, c * P : (c + 1) * P],
                    ones16[:],
                    start=True,
                    stop=True,
                )

            # PSUM -> SBUF fp16 cast (fast DVE op, scale already applied)
            xsT = sp.tile([P, 4], FP16, tag="xsT")
            nc.vector.tensor_copy(xsT[:], p_red[:])

            # fused crow+broadcast+bias: one 5-matmul accumulation group.
            # lhsT = xmean chunk replicated across 128 columns (stride-0
            # free dim), so out[p,n] = sum_k xmean_k @ Wc_k = crow[n] in
            # every partition; a final K=1 matmul vs the bias row adds bc.
            p_out = ps.tile([P, E], FP32, tag="pout")
            for k in range(4):
                nc.tensor.matmul(
                    p_out[:],
                    xsT[:, k : k + 1].broadcast_to([P, P]),
                    wcb[:, k, :],
                    start=(k == 0),
                    stop=(k == 3 and not bias),
                )
            if bias:
                nc.tensor.matmul(
                    p_out[:], ones2[0:1, :], bcr[:], start=False, stop=True
                )
            obuf = sp.tile([P, E], FP16, tag="obuf")
            nc.vector.tensor_copy(obuf[:], p_out[:])

            # 2 stores, each covering 1024 output rows via stride-0 source
            o_t = o_d.rearrange("(p t) e -> p t e", t=N_T)
            src = obuf[:, None, :].broadcast_to([P, 8, E])
            nc.sync.dma_start(o_t[:, 0:8, :], src)
            nc.scalar.dma_start(o_t[:, 8:16, :], src)

            if debug:
                nc.sync.dma_start(dacc_d[:, :], acc[:])
                nc.sync.dma_start(dxst_d[:, :], xsT[:])
                nc.sync.dma_start(dstk_d[0:1, :], bcr[:])
                nc.sync.dma_start(dstk_d[1:2, :], obuf[0:1, :])

    nc.compile()
    _CACHE[key] = nc
    return nc


def _fold_weights(qkv_w, qkv_b, out_w, out_b):
    wv = np.asarray(qkv_w)[:, 2 * E : 3 * E].astype(np.float64)
    ow = np.asarray(out_w).astype(np.float64)
    wc = (wv @ ow).astype(np.float16)
    bc = (np.asarray(qkv_b)[2 * E : 3 * E].astype(np.float64) @ ow
          + np.asarray(out_b)).astype(np.float16)
    return wc, bc


def _run(inputs, trace=False, **kwargs):
    x = np.ascontiguousarray(np.asarray(inputs["x"], dtype=np.float32))
    wc, bc = _fold_weights(
        inputs["qkv_w"], inputs["qkv_b"], inputs["out_w"], inputs["out_b"]
    )
    # zero bias (the common torch-default case) compiles to a no-bias
    # program: numerically exact, one fewer matmul + load
    has_bias = bool(np.any(bc != 0))
    nc = build(bias=has_bias)
    bc0 = np.zeros_like(bc)
    in_maps = []
    for c in range(N_CORES):
        m = {
            "x": np.ascontiguousarray(x[c // 2, (c % 2) * SH : (c % 2 + 1) * SH]),
            "wc": wc,
        }
        if has_bias:
            # the bias must enter the sum exactly once per batch
            m["bc"] = bc if c % 2 == 0 else bc0
        in_maps.append(m)
    res = run_bass_kernel_spmd(
        nc, in_maps, core_ids=list(range(N_CORES)), trace=trace, **kwargs
    )
    # TP-style gather: each core holds a partial of the (row-constant)
    # output; sum the two partials per batch in fp32
    out = np.empty((B, S, E), dtype=np.float32)
    for b in range(B):
        out[b] = res.results[2 * b]["o"].astype(np.float32)
        out[b] += res.results[2 * b + 1]["o"].astype(np.float32)
    return out, res


def kernel(**inputs) -> np.ndarray:
    out, _ = _run(inputs, trace=False)
    return out

